# revision 22
# baseline (speedup 1.0000x reference)
"""Grouped SwiGLU expert MLP (MoE) on 8 Trainium2 NeuronCores.

Problem: sorted_x [32768, 512] f32, tokens pre-sorted by expert into 8 equal
contiguous segments of 4096 tokens; per-expert SwiGLU MLP
    h12 = x_e @ w12[e].T          (4096, 2816)
    h   = silu(h12[:, :1408]) * h12[:, 1408:]
    out = h @ w3[e].T             (4096, 512)

Sharding: pure expert parallelism — core e owns expert e's weights and its
4096-token segment (sliced host-side from expert_starts), so no device-side
collectives are needed; the host concatenates the per-core outputs.

Device layout is feature-major throughout ("contraction dim on partitions"),
which makes both GEMMs transpose-free on chip:
    xt   = x_e.T   [512, 4096]  fp16
    w12t = w12.T   [512, 2816]  fp16
    w3t  = w3.T    [1408, 512]  fp16
    outT = out.T   [512, 4096]  fp16  (host transposes + upcasts back)
GEMM1 produces H12^T tiles [128h, Nt] (PSUM), SwiGLU runs on ACT+DVE into
fp16 H^T tiles, GEMM2 consumes them directly. fp16 operands run the PE at
1 cycle/row (vs 4 for f32) — same speed and footprint as bf16 with a 10-bit
mantissa (8x lower rounding error; inputs here are well inside fp16 range).
Accumulation is always f32 in PSUM; the fp16 output store adds ~2e-4 rel
error (gate is 2e-2) and halves the store bytes + epilogue copy time.
"""

import os

import numpy as np
import ml_dtypes

import concourse.bass as bass
import concourse.mybir as mybir
import concourse.tile as tile
from concourse import bacc
from concourse.bass_utils import run_bass_kernel_spmd

N_CORES = 8
D = 512  # d_model
H = 1408  # hidden
TWOH = 2 * H
TPE = 4096  # tokens per expert
NT = 512  # token block (matmul moving free dim, one PSUM bank in f32)
KD = D // 128  # 4 contraction tiles over d
KH = H // 128  # 11 contraction tiles over h
NB = TPE // NT  # token blocks

F16 = mybir.dt.float16
F32 = mybir.dt.float32
NP_F16 = np.dtype(np.float16)

# Results of a traced run (test harness reads these).
last_exec_time_ns = None
last_trace_path = None


def _build():
    # Bacc (not plain Bass): its compile() pass pipeline legalizes sync
    # waits (>=2 waits per instruction are split into event-sem chains),
    # which this image's walrus requires.
    nc = bacc.Bacc("TRN2", target_bir_lowering=False, debug=False, num_devices=N_CORES)
    xt = nc.dram_tensor("xt", [D, TPE], F16, kind="ExternalInput")
    w12t = nc.dram_tensor("w12t", [D, TWOH], F16, kind="ExternalInput")
    w3t = nc.dram_tensor("w3t", [H, D], F16, kind="ExternalInput")
    outT = nc.dram_tensor("outT", [D, TPE], F16, kind="ExternalOutput")

    # GEMM2 is software-pipelined into the GEMM1/SwiGLU loop with this lag:
    # in iteration hh we issue the GEMM2 matmuls consuming ht[hh - LAG], so
    # the PE never waits on the ACT+DVE SwiGLU chain (~1.3us behind).
    LAG = 2

    # Dummy warm-up matmuls issued while the first input chunks are still in
    # flight: the PE's HAM clock gate needs ~3.4us of sustained activity to
    # lift the cold 4/8 throttle, so burn the DMA head warming it up.  The
    # first x0/g0 chunks land anywhere in ~11.2-12.8us run-to-run, and a PE
    # idle of >~0.7us re-throttles the HAM — after which the next ~3.4us of
    # REAL matmuls run at half rate (~1-2us lost).  46 dummies end ~12.3us
    # (just under the typical arrival); overrunning costs only 56ns per
    # unneeded dummy, so over-provisioning is cheap insurance against the
    # arrival jitter.  A few more are interleaved into block 0's first
    # groups so chunk-landing staggers can't re-throttle mid-block.
    N_WARM = 46
    WARM_IN = {(0, 0): 2, (0, 1): 2, (1, 0): 2, (1, 1): 2}

    with tile.TileContext(nc) as tc:
        with (
            tc.tile_pool(name="weights", bufs=1) as wpool,
            tc.tile_pool(name="xin", bufs=1) as xpool,
            tc.tile_pool(name="ht", bufs=2) as hpool,
            tc.tile_pool(name="swi", bufs=4) as spool,
            tc.tile_pool(name="ot", bufs=4) as opool,
            tc.tile_pool(name="dmy", bufs=1) as dpool,
            tc.tile_pool(name="pg", bufs=2, space=bass.MemorySpace.PSUM) as pgate,
            tc.tile_pool(name="pu", bufs=2, space=bass.MemorySpace.PSUM) as pup,
            tc.tile_pool(name="po", bufs=1, space=bass.MemorySpace.PSUM) as pacc,
        ):
            w12s = wpool.tile([128, KD, TWOH], F16)
            w3s = wpool.tile([128, KH, D], F16)
            xs = xpool.tile([128, KD, TPE], F16)
            dmy = dpool.tile([128, 128], F16)

            nc.vector.memset(dmy[:], 0.0)
            ps_dummy = pgate.tile([128, NT], F32, tag="ps_g")
            for _ in range(N_WARM):
                nc.tensor.matmul(
                    ps_dummy[:, 0:128], dmy[:], dmy[:], start=True, stop=True
                )

            # Each DMA_DIRECT2D costs ~650ns (HWDGE) / ~900ns (SWDGE) of
            # sequencer issue time; each ring executes its transfers serially
            # in FIFO order, and the 16 SDMA engines are chip-shared across
            # all 8 cores' identical schedules (round-robin between busy
            # rings at packet granularity).
            xt_r = xt[:, :].rearrange("(kd p) t -> p kd t", p=128)
            w12_r = w12t[:, :].rearrange("(kd p) h -> p kd h", p=128)
            w3_r = w3t[:, :].rearrange("(kh p) d -> p kh d", p=128)

            def dma_w12(c0, c1, eng):
                eng.dma_start(out=w12s[:, :, c0:c1], in_=w12_r[:, :, c0:c1])

            def dma_x(tb, eng):
                eng.dma_start(
                    out=xs[:, :, tb * NT : (tb + 1) * NT],
                    in_=xt_r[:, :, tb * NT : (tb + 1) * NT],
                )

            def dma_w3(k, eng):
                eng.dma_start(out=w3s[:, k, :], in_=w3_r[:, k, :])

            # A single ring sustains only ~75GB/s on these 256B-line w12
            # chunks, and the g-stream and u-stream each demand ~74GB/s —
            # so they MUST ride separate rings (interleaving both on one
            # ring starves GEMM1, measured as multi-us PE gaps).  Within
            # that constraint, ring slots are filled earliest-deadline-
            # first: every ring's early slots hold early-deadline chunks so
            # no ring prefetches far-future bytes while another core's
            # urgent chunk queues behind chip-shared SDMA engines.
            #   scalar: x0 kd0, kd1, u1, u3, u4, x1   (free until Silu)
            #   sync:   x0 kd2, g0..g10, x2, x3, then per-block stores
            #   gpsimd: x0 kd3, u0, u2, then u5..u10 / w3 k0..k10 woven by
            #           deadline, x4..x7
            nc.scalar.dma_start(out=xs[:, 0, 0:NT], in_=xt_r[:, 0, 0:NT])
            nc.sync.dma_start(out=xs[:, 2, 0:NT], in_=xt_r[:, 2, 0:NT])
            nc.gpsimd.dma_start(out=xs[:, 3, 0:NT], in_=xt_r[:, 3, 0:NT])
            nc.scalar.dma_start(out=xs[:, 1, 0:NT], in_=xt_r[:, 1, 0:NT])

            def dma_u(hh, eng):
                dma_w12(H + hh * 128, H + (hh + 1) * 128, eng)

            # Balanced EDF spread: g1 + odd u's on scalar (free until Silu),
            # g's + odd w3's on sync, even u's + even w3's on gpsimd — every
            # chunk lands >=1.5us before its PE deadline even at worst-case
            # per-ring rates (g1 is tightest and is covered by warm dummies).
            dma_w12(0, 128, nc.sync)  # g0
            dma_u(0, nc.gpsimd)
            dma_w12(128, 256, nc.scalar)  # g1
            dma_u(1, nc.scalar)
            dma_u(2, nc.gpsimd)
            for hh in range(2, 7):
                dma_w12(hh * 128, (hh + 1) * 128, nc.sync)  # g2..g6
            dma_w3(3, nc.sync)
            for hh in range(7, 10):
                dma_w12(hh * 128, (hh + 1) * 128, nc.sync)  # g7..g9
            dma_w3(5, nc.sync)
            dma_w12(10 * 128, 11 * 128, nc.sync)  # g10
            dma_w3(7, nc.sync)
            dma_w3(9, nc.sync)
            dma_u(4, nc.gpsimd)
            dma_w3(0, nc.gpsimd)
            dma_u(6, nc.gpsimd)
            dma_w3(1, nc.gpsimd)
            dma_u(8, nc.gpsimd)
            dma_w3(2, nc.gpsimd)
            dma_u(10, nc.gpsimd)
            for k in (4, 6, 8, 10):
                dma_w3(k, nc.gpsimd)
            dma_u(3, nc.scalar)
            dma_u(5, nc.scalar)
            dma_u(7, nc.scalar)
            dma_u(9, nc.scalar)
            dma_x(1, nc.scalar)
            dma_x(2, nc.sync)
            dma_x(3, nc.sync)
            dma_x(4, nc.gpsimd)
            dma_x(5, nc.gpsimd)
            dma_x(6, nc.gpsimd)
            dma_x(7, nc.gpsimd)

            outT_r = outT[:, :].rearrange("(do p) t -> p do t", p=128)

            for tb in range(NB):
                lag = LAG
                last = tb == NB - 1
                tsl = bass.ts(tb, NT)
                ht = hpool.tile([128, KH, NT], F16)
                acc = [
                    pacc.tile([128, NT], F32, name=f"acc{do}", tag=f"acc{do}")
                    for do in range(KD)
                ]

                def gemm2_step(kh):
                    for do in range(KD):
                        nc.tensor.matmul(
                            acc[do][:],
                            w3s[:, kh, do * 128 : (do + 1) * 128],
                            ht[:, kh, :],
                            start=(kh == 0),
                            stop=(kh == KH - 1),
                        )

                def warm_keep(hh, half):
                    # keep the PE busy across early chunk-landing stalls;
                    # targets acc[0], which gemm2's start=True clears later
                    if tb == 0:
                        for _ in range(WARM_IN.get((hh, half), 0)):
                            nc.tensor.matmul(
                                acc[0][:, 0:128], dmy[:], dmy[:],
                                start=True, stop=True,
                            )

                def gate_group(hh, ps):
                    for kd in range(KD):
                        nc.tensor.matmul(
                            ps[:],
                            w12s[:, kd, hh * 128 : (hh + 1) * 128],
                            xs[:, kd, tsl],
                            start=(kd == 0),
                            stop=(kd == KD - 1),
                        )

                def up_group(hh, ps):
                    for kd in range(KD):
                        nc.tensor.matmul(
                            ps[:],
                            w12s[:, kd, H + hh * 128 : H + (hh + 1) * 128],
                            xs[:, kd, tsl],
                            start=(kd == 0),
                            stop=(kd == KD - 1),
                        )

                def act_pair(hh, ps_g, ps_u):
                    sil = spool.tile([128, NT], F32)
                    nc.scalar.activation(
                        sil[:], ps_g[:], mybir.ActivationFunctionType.Silu
                    )
                    nc.vector.tensor_mul(ht[:, hh, :], sil[:], ps_u[:])

                # Block 0 starts gate0,gate1 -> up0,up1 (both PSUM bufs per
                # pool live at once): that pushes the u0/u1 consumption
                # deadlines ~1.7us later, absorbing DMA arrival jitter on
                # the up-stream during the cold start.
                start_hh = 2 if tb == 0 else 0
                if tb == 0:
                    pg01 = [
                        pgate.tile([128, NT], F32, name=f"pg01_{i}", tag="ps_g")
                        for i in range(2)
                    ]
                    pu01 = [
                        pup.tile([128, NT], F32, name=f"pu01_{i}", tag="ps_u")
                        for i in range(2)
                    ]
                    gate_group(0, pg01[0])
                    warm_keep(0, 0)
                    gate_group(1, pg01[1])
                    warm_keep(0, 1)
                    up_group(0, pu01[0])
                    warm_keep(1, 0)
                    up_group(1, pu01[1])
                    warm_keep(1, 1)
                    act_pair(0, pg01[0], pu01[0])
                    act_pair(1, pg01[1], pu01[1])
                for hh in range(start_hh, KH):
                    ps_g = pgate.tile([128, NT], F32)
                    ps_u = pup.tile([128, NT], F32)
                    gate_group(hh, ps_g)
                    up_group(hh, ps_u)
                    act_pair(hh, ps_g, ps_u)
                    if not last and hh >= lag:
                        gemm2_step(hh - lag)

                ot = opool.tile([128, KD, NT], F16)
                if not last:
                    for kh in range(KH - lag, KH):
                        gemm2_step(kh)
                    # PSUM->SBUF fp16 copies split across ACT and DVE; one
                    # coalesced 512KB output DMA per block on the (now idle)
                    # sync ring.  Overlaps the next block's GEMM1.
                    for do in range(KD):
                        if do % 2 == 0:
                            nc.scalar.copy(ot[:, do, :], acc[do][:])
                        else:
                            nc.vector.tensor_copy(ot[:, do, :], acc[do][:])
                    nc.sync.dma_start(out=outT_r[:, :, tsl], in_=ot[:])
                else:
                    # Last block: do-major GEMM2 so the accumulators finish
                    # staggered ~2.4us apart — each one's copy+store overlaps
                    # the next one's matmul chain, and the kernel's tail after
                    # the very last matmul is only a [128,256] fp16 copy plus
                    # a 64KB store.
                    hn = NT // 2
                    t0, t1 = tb * NT, tb * NT + hn
                    for do in range(KD - 1):
                        for kh in range(KH):
                            nc.tensor.matmul(
                                acc[do][:],
                                w3s[:, kh, do * 128 : (do + 1) * 128],
                                ht[:, kh, :],
                                start=(kh == 0),
                                stop=(kh == KH - 1),
                            )
                        if do % 2 == 0:
                            nc.scalar.copy(ot[:, do, :], acc[do][:])
                        else:
                            nc.vector.tensor_copy(ot[:, do, :], acc[do][:])
                        eng = nc.sync if do % 2 == 0 else nc.scalar
                        eng.dma_start(out=outT_r[:, do, tsl], in_=ot[:, do, :])
                    # The final chains use DIFFERENT PSUM banks (borrowing
                    # acc[2]/acc[1], whose copies completed microseconds
                    # earlier) so no chain's start=True write WAR-stalls on a
                    # previous chunk's copy, and the last chunk shrinks to
                    # [128,128] so the post-matmul tail is one ~250ns cast
                    # plus a 32KB store.
                    do = KD - 1
                    qn = NT // 4
                    chunks = [
                        (0, hn, acc[3]),
                        (hn, hn + qn, acc[2]),
                        (hn + qn, NT, acc[1]),
                    ]
                    for ci, (h0, h1, pt) in enumerate(chunks):
                        for kh in range(KH):
                            nc.tensor.matmul(
                                pt[:, h0:h1],
                                w3s[:, kh, do * 128 : (do + 1) * 128],
                                ht[:, kh, h0:h1],
                                start=(kh == 0),
                                stop=(kh == KH - 1),
                            )
                        if ci == 0:
                            nc.scalar.copy(ot[:, do, h0:h1], pt[:, h0:h1])
                            nc.scalar.dma_start(
                                out=outT_r[:, do, t0 + h0 : t0 + h1],
                                in_=ot[:, do, h0:h1],
                            )
                        else:
                            nc.vector.tensor_copy(
                                ot[:, do, h0:h1], pt[:, h0:h1]
                            )
                            nc.sync.dma_start(
                                out=outT_r[:, do, t0 + h0 : t0 + h1],
                                in_=ot[:, do, h0:h1],
                            )
    nc.compile()
    return nc


_nc_cache = None


def _get_nc():
    global _nc_cache
    if _nc_cache is None:
        _nc_cache = _build()
    return _nc_cache


def kernel(sorted_x, w12, w3, expert_starts, expert_ends):
    global last_exec_time_ns, last_trace_path
    sorted_x = np.asarray(sorted_x)
    w12 = np.asarray(w12)
    w3 = np.asarray(w3)
    starts = np.asarray(expert_starts).astype(np.int64)
    T = sorted_x.shape[0]

    in_maps = []
    for e in range(N_CORES):
        # jax.lax.dynamic_slice clamps the start index the same way
        s = int(min(max(starts[e], 0), T - TPE))
        xe = sorted_x[s : s + TPE]  # (TPE, D) f32
        in_maps.append(
            {
                "xt": np.ascontiguousarray(xe.T).astype(NP_F16),
                "w12t": np.ascontiguousarray(w12[e].T).astype(NP_F16),
                "w3t": np.ascontiguousarray(w3[e].T).astype(NP_F16),
            }
        )

    trace = bool(os.environ.get("BASS_MOE_TRACE"))
    res = run_bass_kernel_spmd(
        _get_nc(), in_maps, core_ids=list(range(N_CORES)), trace=trace
    )
    if trace:
        last_exec_time_ns = res.exec_time_ns
        iat = res.instructions_and_trace
        last_trace_path = iat[1] if iat else None

    out = np.empty((N_CORES * TPE, D), dtype=np.float32)
    for e in range(N_CORES):
        out[e * TPE : (e + 1) * TPE] = res.results[e]["outT"].T.astype(np.float32)
    return out


# revision 23
# speedup vs baseline: 1.0179x; 1.0179x over previous
"""Grouped SwiGLU expert MLP (MoE) on 8 Trainium2 NeuronCores.

Problem: sorted_x [32768, 512] f32, tokens pre-sorted by expert into 8 equal
contiguous segments of 4096 tokens; per-expert SwiGLU MLP
    h12 = x_e @ w12[e].T          (4096, 2816)
    h   = silu(h12[:, :1408]) * h12[:, 1408:]
    out = h @ w3[e].T             (4096, 512)

Sharding: pure expert parallelism — core e owns expert e's weights and its
4096-token segment (sliced host-side from expert_starts), so no device-side
collectives are needed; the host concatenates the per-core outputs.

Device layout is feature-major throughout ("contraction dim on partitions"),
which makes both GEMMs transpose-free on chip:
    xt   = x_e.T   [512, 4096]  fp16
    w12t = w12.T   [512, 2816]  fp16
    w3t  = w3.T    [1408, 512]  fp16
    outT = out.T   [512, 4096]  fp16  (host transposes + upcasts back)
GEMM1 produces H12^T tiles [128h, Nt] (PSUM), SwiGLU runs on ACT+DVE into
fp16 H^T tiles, GEMM2 consumes them directly. fp16 operands run the PE at
1 cycle/row (vs 4 for f32) — same speed and footprint as bf16 with a 10-bit
mantissa (8x lower rounding error; inputs here are well inside fp16 range).
Accumulation is always f32 in PSUM; the fp16 output store adds ~2e-4 rel
error (gate is 2e-2) and halves the store bytes + epilogue copy time.
"""

import os

import numpy as np
import ml_dtypes

import concourse.bass as bass
import concourse.mybir as mybir
import concourse.tile as tile
from concourse import bacc
from concourse.bass_utils import run_bass_kernel_spmd

N_CORES = 8
D = 512  # d_model
H = 1408  # hidden
TWOH = 2 * H
TPE = 4096  # tokens per expert
NT = 512  # token block (matmul moving free dim, one PSUM bank in f32)
KD = D // 128  # 4 contraction tiles over d
KH = H // 128  # 11 contraction tiles over h
NB = TPE // NT  # token blocks

F16 = mybir.dt.float16
F32 = mybir.dt.float32
NP_F16 = np.dtype(np.float16)

# Results of a traced run (test harness reads these).
last_exec_time_ns = None
last_trace_path = None


def _build():
    # Bacc (not plain Bass): its compile() pass pipeline legalizes sync
    # waits (>=2 waits per instruction are split into event-sem chains),
    # which this image's walrus requires.
    nc = bacc.Bacc("TRN2", target_bir_lowering=False, debug=False, num_devices=N_CORES)
    xt = nc.dram_tensor("xt", [D, TPE], F16, kind="ExternalInput")
    w12t = nc.dram_tensor("w12t", [D, TWOH], F16, kind="ExternalInput")
    w3t = nc.dram_tensor("w3t", [H, D], F16, kind="ExternalInput")
    outT = nc.dram_tensor("outT", [D, TPE], F16, kind="ExternalOutput")

    # GEMM2 is software-pipelined into the GEMM1/SwiGLU loop with this lag:
    # in iteration hh we issue the GEMM2 matmuls consuming ht[hh - LAG], so
    # the PE never waits on the ACT+DVE SwiGLU chain (~1.3us behind).
    LAG = 2

    # Dummy warm-up matmuls issued while the first input chunks are still in
    # flight: the PE's HAM clock gate needs ~3.4us of sustained activity to
    # lift the cold 4/8 throttle, so burn the DMA head warming it up.  The
    # first x0/g0 chunks land anywhere in ~11.2-12.8us run-to-run, and a PE
    # idle of >~0.7us re-throttles the HAM — after which the next ~3.4us of
    # REAL matmuls run at half rate (~1-2us lost).  46 dummies end ~12.3us
    # (just under the typical arrival); overrunning costs only 56ns per
    # unneeded dummy, so over-provisioning is cheap insurance against the
    # arrival jitter.  A few more are interleaved into block 0's first
    # groups so chunk-landing staggers can't re-throttle mid-block.
    N_WARM = 46
    WARM_IN = {(0, 0): 2, (0, 1): 2, (1, 0): 2, (1, 1): 2}

    with tile.TileContext(nc) as tc:
        with (
            tc.tile_pool(name="weights", bufs=1) as wpool,
            tc.tile_pool(name="xin", bufs=1) as xpool,
            tc.tile_pool(name="ht", bufs=2) as hpool,
            tc.tile_pool(name="swi", bufs=4) as spool,
            tc.tile_pool(name="ot", bufs=4) as opool,
            tc.tile_pool(name="dmy", bufs=1) as dpool,
            tc.tile_pool(name="pg", bufs=2, space=bass.MemorySpace.PSUM) as pgate,
            tc.tile_pool(name="pu", bufs=2, space=bass.MemorySpace.PSUM) as pup,
            tc.tile_pool(name="po", bufs=1, space=bass.MemorySpace.PSUM) as pacc,
        ):
            w12s = wpool.tile([128, KD, TWOH], F16)
            w3s = wpool.tile([128, KH, D], F16)
            xs = xpool.tile([128, KD, TPE], F16)
            dmy = dpool.tile([128, 128], F16)

            nc.vector.memset(dmy[:], 0.0)
            ps_dummy = pgate.tile([128, NT], F32, tag="ps_g")
            for _ in range(N_WARM):
                nc.tensor.matmul(
                    ps_dummy[:, 0:128], dmy[:], dmy[:], start=True, stop=True
                )

            # Each DMA_DIRECT2D costs ~650ns (HWDGE) / ~900ns (SWDGE) of
            # sequencer issue time; each ring executes its transfers serially
            # in FIFO order, and the 16 SDMA engines are chip-shared across
            # all 8 cores' identical schedules (round-robin between busy
            # rings at packet granularity).
            xt_r = xt[:, :].rearrange("(kd p) t -> p kd t", p=128)
            w12_r = w12t[:, :].rearrange("(kd p) h -> p kd h", p=128)
            w3_r = w3t[:, :].rearrange("(kh p) d -> p kh d", p=128)

            def dma_w12(c0, c1, eng):
                eng.dma_start(out=w12s[:, :, c0:c1], in_=w12_r[:, :, c0:c1])

            def dma_x(tb, eng):
                eng.dma_start(
                    out=xs[:, :, tb * NT : (tb + 1) * NT],
                    in_=xt_r[:, :, tb * NT : (tb + 1) * NT],
                )

            def dma_w3(k, eng):
                eng.dma_start(out=w3s[:, k, :], in_=w3_r[:, k, :])

            # A single ring sustains only ~75GB/s on these 256B-line w12
            # chunks, and the g-stream and u-stream each demand ~74GB/s —
            # so they MUST ride separate rings (interleaving both on one
            # ring starves GEMM1, measured as multi-us PE gaps).  Within
            # that constraint, ring slots are filled earliest-deadline-
            # first: every ring's early slots hold early-deadline chunks so
            # no ring prefetches far-future bytes while another core's
            # urgent chunk queues behind chip-shared SDMA engines.
            #   scalar: x0 kd0, kd1, u1, u3, u4, x1   (free until Silu)
            #   sync:   x0 kd2, g0..g10, x2, x3, then per-block stores
            #   gpsimd: x0 kd3, u0, u2, then u5..u10 / w3 k0..k10 woven by
            #           deadline, x4..x7
            nc.scalar.dma_start(out=xs[:, 0, 0:NT], in_=xt_r[:, 0, 0:NT])
            nc.sync.dma_start(out=xs[:, 2, 0:NT], in_=xt_r[:, 2, 0:NT])
            nc.gpsimd.dma_start(out=xs[:, 3, 0:NT], in_=xt_r[:, 3, 0:NT])
            nc.scalar.dma_start(out=xs[:, 1, 0:NT], in_=xt_r[:, 1, 0:NT])

            def dma_u(hh, eng):
                dma_w12(H + hh * 128, H + (hh + 1) * 128, eng)

            # Balanced spread (best measured): the g-stream rides sync
            # alone (~74GB/s demand vs ~75GB/s single-ring line rate); the
            # u-stream is split between scalar's head (u1,u3,u4,u5 — the
            # ACT sequencer is free until its first Silu) and gpsimd, whose
            # slower SWDGE ring also weaves in the w3 kh chunks two slots
            # ahead of their pipelined-GEMM2 deadlines.
            dma_w12(0, 128, nc.sync)  # g0
            dma_u(0, nc.gpsimd)
            dma_u(1, nc.scalar)
            for hh in range(1, KH):
                dma_w12(hh * 128, (hh + 1) * 128, nc.sync)  # g_hh
            dma_u(2, nc.gpsimd)
            dma_u(3, nc.scalar)
            dma_u(4, nc.scalar)
            dma_u(5, nc.scalar)
            for j in range(5):  # w3k0/u6, w3k1/u7, ... w3k4/u10
                dma_w3(j, nc.gpsimd)
                dma_u(6 + j, nc.gpsimd)
            for k in range(5, KH):
                dma_w3(k, nc.gpsimd)
            dma_x(1, nc.scalar)
            dma_x(2, nc.sync)
            dma_x(3, nc.sync)
            dma_x(4, nc.gpsimd)
            dma_x(5, nc.gpsimd)
            dma_x(6, nc.gpsimd)
            dma_x(7, nc.gpsimd)

            outT_r = outT[:, :].rearrange("(do p) t -> p do t", p=128)

            for tb in range(NB):
                lag = LAG
                last = tb == NB - 1
                tsl = bass.ts(tb, NT)
                ht = hpool.tile([128, KH, NT], F16)
                acc = [
                    pacc.tile([128, NT], F32, name=f"acc{do}", tag=f"acc{do}")
                    for do in range(KD)
                ]

                def gemm2_step(kh):
                    for do in range(KD):
                        nc.tensor.matmul(
                            acc[do][:],
                            w3s[:, kh, do * 128 : (do + 1) * 128],
                            ht[:, kh, :],
                            start=(kh == 0),
                            stop=(kh == KH - 1),
                        )

                def warm_keep(hh, half):
                    # keep the PE busy across early chunk-landing stalls;
                    # targets acc[0], which gemm2's start=True clears later
                    if tb == 0:
                        for _ in range(WARM_IN.get((hh, half), 0)):
                            nc.tensor.matmul(
                                acc[0][:, 0:128], dmy[:], dmy[:],
                                start=True, stop=True,
                            )

                def gate_group(hh, ps):
                    for kd in range(KD):
                        nc.tensor.matmul(
                            ps[:],
                            w12s[:, kd, hh * 128 : (hh + 1) * 128],
                            xs[:, kd, tsl],
                            start=(kd == 0),
                            stop=(kd == KD - 1),
                        )

                def up_group(hh, ps):
                    for kd in range(KD):
                        nc.tensor.matmul(
                            ps[:],
                            w12s[:, kd, H + hh * 128 : H + (hh + 1) * 128],
                            xs[:, kd, tsl],
                            start=(kd == 0),
                            stop=(kd == KD - 1),
                        )

                def act_pair(hh, ps_g, ps_u):
                    sil = spool.tile([128, NT], F32)
                    nc.scalar.activation(
                        sil[:], ps_g[:], mybir.ActivationFunctionType.Silu
                    )
                    nc.vector.tensor_mul(ht[:, hh, :], sil[:], ps_u[:])

                # Block 0 starts gate0,gate1 -> up0,up1 (both PSUM bufs per
                # pool live at once): that pushes the u0/u1 consumption
                # deadlines ~1.7us later, absorbing DMA arrival jitter on
                # the up-stream during the cold start.
                start_hh = 2 if tb == 0 else 0
                if tb == 0:
                    pg01 = [
                        pgate.tile([128, NT], F32, name=f"pg01_{i}", tag="ps_g")
                        for i in range(2)
                    ]
                    pu01 = [
                        pup.tile([128, NT], F32, name=f"pu01_{i}", tag="ps_u")
                        for i in range(2)
                    ]
                    gate_group(0, pg01[0])
                    warm_keep(0, 0)
                    gate_group(1, pg01[1])
                    warm_keep(0, 1)
                    up_group(0, pu01[0])
                    warm_keep(1, 0)
                    up_group(1, pu01[1])
                    warm_keep(1, 1)
                    act_pair(0, pg01[0], pu01[0])
                    act_pair(1, pg01[1], pu01[1])
                for hh in range(start_hh, KH):
                    ps_g = pgate.tile([128, NT], F32)
                    ps_u = pup.tile([128, NT], F32)
                    gate_group(hh, ps_g)
                    up_group(hh, ps_u)
                    act_pair(hh, ps_g, ps_u)
                    if not last and hh >= lag:
                        gemm2_step(hh - lag)

                ot = opool.tile([128, KD, NT], F16)
                if not last:
                    for kh in range(KH - lag, KH):
                        gemm2_step(kh)
                    # PSUM->SBUF fp16 copies split across ACT and DVE; one
                    # coalesced 512KB output DMA per block on the (now idle)
                    # sync ring.  Overlaps the next block's GEMM1.
                    for do in range(KD):
                        if do % 2 == 0:
                            nc.scalar.copy(ot[:, do, :], acc[do][:])
                        else:
                            nc.vector.tensor_copy(ot[:, do, :], acc[do][:])
                    nc.sync.dma_start(out=outT_r[:, :, tsl], in_=ot[:])
                else:
                    # Last block: do-major GEMM2 so the accumulators finish
                    # staggered ~2.4us apart — each one's copy+store overlaps
                    # the next one's matmul chain, and the kernel's tail after
                    # the very last matmul is only a [128,256] fp16 copy plus
                    # a 64KB store.
                    hn = NT // 2
                    t0, t1 = tb * NT, tb * NT + hn
                    for do in range(KD - 1):
                        for kh in range(KH):
                            nc.tensor.matmul(
                                acc[do][:],
                                w3s[:, kh, do * 128 : (do + 1) * 128],
                                ht[:, kh, :],
                                start=(kh == 0),
                                stop=(kh == KH - 1),
                            )
                        if do % 2 == 0:
                            nc.scalar.copy(ot[:, do, :], acc[do][:])
                        else:
                            nc.vector.tensor_copy(ot[:, do, :], acc[do][:])
                        eng = nc.sync if do % 2 == 0 else nc.scalar
                        eng.dma_start(out=outT_r[:, do, tsl], in_=ot[:, do, :])
                    # The final chains use DIFFERENT PSUM banks (borrowing
                    # acc[2]/acc[1], whose copies completed microseconds
                    # earlier) so no chain's start=True write WAR-stalls on a
                    # previous chunk's copy, and the last chunk shrinks to
                    # [128,128] so the post-matmul tail is one ~250ns cast
                    # plus a 32KB store.
                    do = KD - 1
                    qn = NT // 4
                    chunks = [
                        (0, hn, acc[3]),
                        (hn, hn + qn, acc[2]),
                        (hn + qn, NT, acc[1]),
                    ]
                    for ci, (h0, h1, pt) in enumerate(chunks):
                        for kh in range(KH):
                            nc.tensor.matmul(
                                pt[:, h0:h1],
                                w3s[:, kh, do * 128 : (do + 1) * 128],
                                ht[:, kh, h0:h1],
                                start=(kh == 0),
                                stop=(kh == KH - 1),
                            )
                        if ci == 0:
                            nc.scalar.copy(ot[:, do, h0:h1], pt[:, h0:h1])
                            nc.scalar.dma_start(
                                out=outT_r[:, do, t0 + h0 : t0 + h1],
                                in_=ot[:, do, h0:h1],
                            )
                        else:
                            nc.vector.tensor_copy(
                                ot[:, do, h0:h1], pt[:, h0:h1]
                            )
                            nc.sync.dma_start(
                                out=outT_r[:, do, t0 + h0 : t0 + h1],
                                in_=ot[:, do, h0:h1],
                            )
    nc.compile()
    return nc


_nc_cache = None


def _get_nc():
    global _nc_cache
    if _nc_cache is None:
        _nc_cache = _build()
    return _nc_cache


def kernel(sorted_x, w12, w3, expert_starts, expert_ends):
    global last_exec_time_ns, last_trace_path
    sorted_x = np.asarray(sorted_x)
    w12 = np.asarray(w12)
    w3 = np.asarray(w3)
    starts = np.asarray(expert_starts).astype(np.int64)
    T = sorted_x.shape[0]

    in_maps = []
    for e in range(N_CORES):
        # jax.lax.dynamic_slice clamps the start index the same way
        s = int(min(max(starts[e], 0), T - TPE))
        xe = sorted_x[s : s + TPE]  # (TPE, D) f32
        in_maps.append(
            {
                "xt": np.ascontiguousarray(xe.T).astype(NP_F16),
                "w12t": np.ascontiguousarray(w12[e].T).astype(NP_F16),
                "w3t": np.ascontiguousarray(w3[e].T).astype(NP_F16),
            }
        )

    trace = bool(os.environ.get("BASS_MOE_TRACE"))
    res = run_bass_kernel_spmd(
        _get_nc(), in_maps, core_ids=list(range(N_CORES)), trace=trace
    )
    if trace:
        last_exec_time_ns = res.exec_time_ns
        iat = res.instructions_and_trace
        last_trace_path = iat[1] if iat else None

    out = np.empty((N_CORES * TPE, D), dtype=np.float32)
    for e in range(N_CORES):
        out[e * TPE : (e + 1) * TPE] = res.results[e]["outT"].T.astype(np.float32)
    return out
